# revision 19
# baseline (speedup 1.0000x reference)
"""Trainium2 Bass kernel for nn_Decoder (embed -> LSTM -> vocab projection).

v2 layout (8 NeuronCores, single SPMD NEFF):
  - Host: embedding gather + concat -> lstm_in; pre-transpose weights.
  - gx GEMM in rows-layout: gx[320,4096] = x @ W_ih^T + b (x stationary,
    W_ih^T moving, bias added on PSUM eviction).
  - LSTM recurrence, data-parallel over batch (32/core): gates[32,4096]
    computed with h_T as the stationary operand (64 N=512 matmuls/step) and
    the gx contribution folded in via an identity-matmul partition-select.
    ScalarE applies sigmoid/tanh straight from PSUM. h is re-transposed
    each step on the PE (h_T feeds the next step + the FC lhsT).
  - Per-timestep AllGather of h_T (runs on TOPSP, overlapped with compute).
  - FC vocab-sharded: logits[2560,3750] = hs @ fc_W_shard^T + fc_b, rows
    processed in 2 halves to fit SBUF; 1280 N<=512 fp32r matmuls.
  - Host: undo row permutation, concat vocab shards.

All matmuls are float32r (tf32-class, ~1.5e-4): raw fp32 bits are DMA'd
directly into float32r tensors (no on-device rounding pass needed).
"""
import ml_dtypes
import numpy as np
import jax
from jax.sharding import Mesh, PartitionSpec
from jax.experimental.shard_map import shard_map

import concourse.bass as bass
import concourse.mybir as mybir
import concourse.tile as tile
from concourse import bacc
from concourse.bass2jax import _bass_exec_p, install_neuronx_cc_hook, partition_id_tensor
from concourse.masks import make_identity

P = 128
NCORES = 8
B, T, FEAT, EMB, HID, VOCAB = 256, 10, 512, 512, 1024, 30000
DIN = FEAT + EMB          # 1024
G = 4 * HID               # 4096
BL = B // NCORES          # 32 batches per core
RL = BL * T               # 320 rows per core (t-major: r = t*BL + b)
RA = RL * NCORES          # 2560 rows total
VL = VOCAB // NCORES      # 3750 vocab per core
KT = DIN // P             # 8 contraction tiles
NCH = G // 512            # 8 gate column chunks
F32 = mybir.dt.float32
F32R = mybir.dt.float32r
BF16 = mybir.dt.bfloat16
Act = mybir.ActivationFunctionType

_CACHE = {}


def _build_nc():
    nc = bacc.Bacc("TRN2", target_bir_lowering=False, debug=False, num_devices=NCORES)
    x_T = nc.dram_tensor("x_T", [DIN, RL], BF16, kind="ExternalInput").ap()
    w_ih_T = nc.dram_tensor("w_ih_T", [DIN, G], BF16, kind="ExternalInput").ap()
    w_hh_T = nc.dram_tensor("w_hh_T", [HID, G], BF16, kind="ExternalInput").ap()
    bias_rep = nc.dram_tensor("bias_rep", [P, G], F32, kind="ExternalInput").ap()
    fc_wT = nc.dram_tensor("fc_wT", [HID, VL], BF16, kind="ExternalInput").ap()
    fc_b_rep = nc.dram_tensor("fc_b_rep", [P, VL], F32, kind="ExternalInput").ap()
    logits = nc.dram_tensor("logits", [RA, VL], F32, kind="ExternalOutput").ap()

    MT_X = [(0, 128), (128, 128), (256, 64)]  # (row0, rows) m-tiles of 320
    WINS = [(0, 1024), (1024, 1024), (2048, 1024), (3072, 678)]
    NORD = [0, 2, 4, 6, 1, 3, 5, 7]  # gate chunks: h-half0 first, then half1

    with tile.TileContext(nc) as tc:
        with tc.tile_pool(name="dram", bufs=1, space="DRAM") as dram_pool:
            hs_dram = dram_pool.tile([T, HID, BL], BF16)
            ag_outs = [dram_pool.tile([NCORES, HID, BL], BF16,
                                      addr_space="Shared", name=f"ag_{t}")
                       for t in range(T)]
            gx_dram = dram_pool.tile([3, P, G], BF16)

            with tc.tile_pool(name="persist", bufs=1) as persist, \
                 tc.tile_pool(name="phD", bufs=1) as phD, \
                 tc.tile_pool(name="hsT_pool", bufs=2) as hsT_pool, \
                 tc.tile_pool(name="fcw_pool", bufs=2) as fcw_pool, \
                 tc.tile_pool(name="fc_out", bufs=2) as fc_out:
                ident_f = persist.tile([P, P], F32)
                make_identity(nc, ident_f[:])
                ident_b = persist.tile([P, P], BF16)
                nc.vector.tensor_copy(ident_b[:], ident_f[:])
                gx_t0 = persist.tile([32, G], BF16)
                fcb_sb = phD.tile([P, VL], F32)
                GSZ = [512, 512, 256]   # rows per t-group (t0-3, t4-7, t8-9)
                GT0 = [0, 4, 8]
                hsT_tiles = {}
                for rh in range(2):
                    for g in range(3):
                        hsT_tiles[rh, g] = hsT_pool.tile(
                            [P, KT, GSZ[g]], BF16,
                            name=f"hsT_{rh}_{g}", tag=f"hsT_{g}")

                GB = [0, 512, 1024]

                def fc_block(rh, v0, vn, fw, g, ml, psum_pool, tagsfx=""):
                    row0 = rh * (RA // 2) + GB[g] + ml * P
                    hsT_sb = hsT_tiles[rh, g]
                    for n0 in range(0, vn, 512):
                        nsz = min(512, vn - n0)
                        ps = psum_pool.tile(
                            [P, 512], F32,
                            name=f"fps{tagsfx}_{rh}_{v0}_{g}_{ml}_{n0}",
                            tag=f"fps{tagsfx}", bufs=2 if tagsfx else None)
                        for k in range(KT):
                            nc.tensor.matmul(
                                ps[:, 0:nsz],
                                hsT_sb[:, k, ml * P:(ml + 1) * P],
                                fw[:, k, n0:n0 + nsz],
                                start=(k == 0), stop=(k == KT - 1))
                        ot = fc_out.tile(
                            [P, 512], F32,
                            name=f"fo_{rh}_{v0}_{g}_{ml}_{n0}", tag="fo")
                        nc.vector.tensor_add(
                            ot[:, 0:nsz], ps[:, 0:nsz],
                            fcb_sb[:, v0 + n0:v0 + n0 + nsz])
                        nc.sync.dma_start(
                            logits[row0:row0 + P, v0 + n0:v0 + n0 + nsz],
                            ot[:, 0:nsz])

                # ---- Phase A: gx = x @ W_ih^T + b   (rows x gates, bf16) ----
                with tc.tile_pool(name="phA", bufs=1) as phA, \
                     tc.tile_pool(name="wih_pool", bufs=3) as wih_pool, \
                     tc.tile_pool(name="gx_stage", bufs=4) as gx_stage, \
                     tc.tile_pool(name="gx_psum", bufs=4, space="PSUM") as gx_psum:
                    x_sb = phA.tile([P, KT, RL], BF16)
                    for k in range(KT):
                        nc.scalar.dma_start(
                            x_sb[:, k, :], x_T[k * P:(k + 1) * P, :])
                    bias_sb = phA.tile([P, G], F32)
                    nc.scalar.dma_start(bias_sb[:], bias_rep)
                    for n in range(NCH):
                        wt = wih_pool.tile([P, KT, 512], BF16,
                                           name=f"wih_{n}", tag="wih")
                        for kk in range(0, KT, 2):
                            eng = nc.sync if (n + kk // 2) % 2 == 0 else nc.scalar
                            eng.dma_start(
                                wt[:, kk:kk + 2, :],
                                w_ih_T[:, n * 512:(n + 1) * 512].rearrange(
                                    "(k p) v -> p k v", p=P)[:, kk:kk + 2, :])
                        for mi, (r0, rn) in enumerate(MT_X):
                            ps = gx_psum.tile([P, 512], F32,
                                              name=f"gxps_{n}_{mi}", tag="gxps")
                            for k in range(KT):
                                nc.tensor.matmul(
                                    ps[0:rn, :], x_sb[:, k, r0:r0 + rn],
                                    wt[:, k, :],
                                    start=(k == 0), stop=(k == KT - 1))
                            gt = gx_stage.tile([P, 512], BF16,
                                               name=f"gxs_{n}_{mi}", tag="gxs")
                            nc.vector.tensor_add(
                                gt[0:rn, :],
                                ps[0:rn, :], bias_sb[0:rn, n * 512:(n + 1) * 512])
                            if mi == 0:
                                # keep t=0's rows on-chip (skips DRAM roundtrip)
                                nc.vector.tensor_copy(
                                    gx_t0[:, n * 512:(n + 1) * 512], gt[0:32, :])
                            nc.scalar.dma_start(
                                gx_dram[mi, 0:rn, n * 512:(n + 1) * 512], gt[0:rn, :])

                nc.scalar.dma_start(fcb_sb[:], fc_b_rep)

                # ---- Phase B: LSTM recurrence (bf16) ----
                with tc.tile_pool(name="phB", bufs=1) as phB, \
                     tc.tile_pool(name="whh_pool", bufs=1) as whh_pool, \
                     tc.tile_pool(name="gxb_pool", bufs=2) as gxb_pool, \
                     tc.tile_pool(name="gch_psum", bufs=4, space="PSUM") as gch_psum, \
                     tc.tile_pool(name="tp_psum", bufs=2, space="PSUM") as tp_psum, \
                     tc.tile_pool(name="step_pool", bufs=1) as step_pool:
                    whh_sb = whh_pool.tile([P, KT, G], BF16)
                    for n in range(NCH):
                        for kk in range(0, KT, 4):
                            nc.gpsimd.dma_start(
                                whh_sb[:, kk:kk + 4, n * 512:(n + 1) * 512],
                                w_hh_T[:, n * 512:(n + 1) * 512].rearrange(
                                    "(k p) v -> p k v", p=P)[:, kk:kk + 4, :])
                    h_Tb = phB.tile([P, KT, BL], BF16)
                    c_sb = phB.tile([32, HID], F32)
                    fw0 = fcw_pool.tile([P, KT, 1024], BF16, name="fcw_w0", tag="fcw")
                    for kk in range(0, KT, 4):
                        nc.sync.dma_start(
                            fw0[:, kk:kk + 4, :],
                            fc_wT[:, 0:1024].rearrange(
                                "(k p) v -> p k v", p=P)[:, kk:kk + 4, :])

                    for t in range(T):
                        mt, j = t // 4, t % 4
                        if t > 0:
                            gxt = gxb_pool.tile([32, G], BF16,
                                                name=f"gxt_{t}", tag="gxt")
                            nc.scalar.dma_start(
                                gxt[:], gx_dram[mt, 32 * j:32 * j + 32, :])
                        gates4 = step_pool.tile([32, 4, HID], F32,
                                                name=f"gates_{t}", tag="gates")
                        tmp = step_pool.tile([32, HID], F32, name=f"tmp_{t}", tag="tmp")
                        th = step_pool.tile([32, HID], F32, name=f"th_{t}", tag="tmp")
                        h_sb = step_pool.tile([32, HID], BF16, name=f"h_{t}", tag="h")

                        def gate_chunk(n):
                            g4, half = n // 2, n % 2
                            dst = gates4[:, g4, half * 512:(half + 1) * 512]
                            if t == 0:
                                nc.scalar.activation(
                                    dst, gx_t0[:, n * 512:(n + 1) * 512],
                                    Act.Tanh if g4 == 2 else Act.Sigmoid)
                                return
                            ps = gch_psum.tile([32, 512], F32,
                                               name=f"gps_{t}_{n}", tag="gps")
                            nc.tensor.matmul(
                                ps[:], ident_b[0:32, 0:32],
                                gxt[:, n * 512:(n + 1) * 512],
                                start=True, stop=False)
                            for k in range(KT):
                                nc.tensor.matmul(
                                    ps[:], h_Tb[:, k, :],
                                    whh_sb[:, k, n * 512:(n + 1) * 512],
                                    start=False, stop=(k == KT - 1))
                            nc.scalar.activation(
                                dst, ps[:], Act.Tanh if g4 == 2 else Act.Sigmoid)

                        def half_elemwise(half):
                            sl = slice(half * 512, (half + 1) * 512)
                            nc.vector.tensor_mul(tmp[:, sl], gates4[:, 0, sl],
                                                 gates4[:, 2, sl])
                            if t == 0:
                                nc.vector.tensor_copy(c_sb[:, sl], tmp[:, sl])
                            else:
                                nc.vector.tensor_mul(c_sb[:, sl], gates4[:, 1, sl],
                                                     c_sb[:, sl])
                                nc.vector.tensor_add(c_sb[:, sl], c_sb[:, sl],
                                                     tmp[:, sl])
                            nc.scalar.activation(th[:, sl], c_sb[:, sl], Act.Tanh)
                            nc.vector.tensor_mul(h_sb[:, sl], gates4[:, 3, sl],
                                                 th[:, sl])

                        for n in NORD[:4]:
                            gate_chunk(n)
                        half_elemwise(0)
                        for n in NORD[4:]:
                            gate_chunk(n)
                        half_elemwise(1)
                        for k in range(KT):
                            tp = tp_psum.tile([P, 32], BF16,
                                              name=f"tp_{t}_{k}", tag="tp")
                            nc.tensor.transpose(
                                tp[:], h_sb[:, k * P:(k + 1) * P], ident_b[0:32, 0:32])
                            nc.vector.tensor_copy(h_Tb[:, k, :], tp[:])
                        nc.scalar.dma_start(
                            hs_dram[t].rearrange("(k p) b -> p k b", p=P), h_Tb[:])
                        nc.gpsimd.collective_compute(
                            "AllGather", mybir.AluOpType.bypass,
                            replica_groups=[list(range(NCORES))],
                            ins=[hs_dram[t].opt()], outs=[ag_outs[t].opt()])
                        tg = 0 if t < 4 else (1 if t < 8 else 2)
                        ntg = 4 if tg < 2 else 2
                        for a in range(NCORES):
                            rh = a // 4
                            r0 = (a % 4) * ntg * BL + (t - GT0[tg]) * BL
                            nc.gpsimd.dma_start(
                                hsT_tiles[rh, tg][:, :, r0:r0 + BL],
                                ag_outs[t][a].rearrange("(k p) b -> p k b", p=P))
                        # backfill PE stalls with early FC work (fw0 resident)
                        ILV = {6: [(0, 0)], 7: [(0, 1)],
                               8: [(0, 2), (0, 3)], 9: [(1, 0), (1, 1)]}
                        for (g, ml) in ILV.get(t, []):
                            fc_block(0, 0, 1024, fw0, g, ml, gch_psum, tagsfx="i")

                # ---- Phase D: FC; t-groups 0/1 first, g2 (needs AG_9) last ----
                with tc.tile_pool(name="fc_psum", bufs=6, space="PSUM") as fc_psum:
                    def load_fw(tag_name, v0, vn):
                        fw = fcw_pool.tile([P, KT, 1024], BF16,
                                           name=tag_name, tag="fcw")
                        for kk in range(0, KT, 4):
                            nc.sync.dma_start(
                                fw[:, kk:kk + 4, 0:vn],
                                fc_wT[:, v0:v0 + vn].rearrange(
                                    "(k p) v -> p k v", p=P)[:, kk:kk + 4, :])
                        return fw

                    for rh in range(2):
                        for wi, (v0, vn) in enumerate(WINS):
                            fw = fw0 if (rh == 0 and wi == 0) \
                                else load_fw(f"fcw_{rh}_{v0}", v0, vn)
                            done = {(0, 0), (0, 1), (0, 2), (0, 3),
                                    (1, 0), (1, 1)} if (rh == 0 and wi == 0) else set()
                            for g in (0, 1):
                                for ml in range(4):
                                    if (g, ml) in done or (g == 1 and ml >= 4):
                                        continue
                                    fc_block(rh, v0, vn, fw, g, ml, fc_psum)
                    for rh in range(2):
                        for wi, (v0, vn) in enumerate(WINS):
                            fw = load_fw(f"fcwg2_{rh}_{v0}", v0, vn)
                            for ml in range(2):
                                fc_block(rh, v0, vn, fw, 2, ml, fc_psum)
    nc.compile()
    return nc


def _build_sharded(nc, n_cores=NCORES):
    install_neuronx_cc_hook()
    partition_name = nc.partition_id_tensor.name if nc.partition_id_tensor else None
    in_names, out_names, out_avals, zero_shapes = [], [], [], []
    for alloc in nc.m.functions[0].allocations:
        if not isinstance(alloc, mybir.MemoryLocationSet):
            continue
        name = alloc.memorylocations[0].name
        if alloc.kind == "ExternalInput":
            if name != partition_name:
                in_names.append(name)
        elif alloc.kind == "ExternalOutput":
            out_names.append(name)
            shape = tuple(alloc.tensor_shape)
            dtype = mybir.dt.np(alloc.dtype)
            out_avals.append(jax.core.ShapedArray(shape, dtype))
            zero_shapes.append((shape, dtype))
    n_params = len(in_names)
    n_outs = len(out_avals)
    all_in_names = list(in_names) + list(out_names)
    if partition_name is not None:
        all_in_names.append(partition_name)
    donate = tuple(range(n_params, n_params + n_outs))

    def _body(*args):
        operands = list(args)
        if partition_name is not None:
            operands.append(partition_id_tensor())
        outs = _bass_exec_p.bind(
            *operands,
            out_avals=tuple(out_avals),
            in_names=tuple(all_in_names),
            out_names=tuple(out_names),
            lowering_input_output_aliases=(),
            sim_require_finite=True,
            sim_require_nnan=True,
            nc=nc,
        )
        return tuple(outs)

    devices = jax.devices("axon")[:n_cores]
    mesh = Mesh(np.asarray(devices), ("core",))
    in_specs = (PartitionSpec("core"),) * (n_params + n_outs)
    out_specs = (PartitionSpec("core"),) * len(out_names)
    sharded = jax.jit(
        shard_map(_body, mesh=mesh, in_specs=in_specs, out_specs=out_specs,
                  check_rep=False),
        donate_argnums=donate, keep_unused=True)

    def run(in_maps):
        concat_in = [
            np.concatenate([np.asarray(m[name]) for m in in_maps], axis=0)
            for name in in_names
        ]
        concat_zeros = [np.zeros((n_cores * s[0], *s[1:]), d) for s, d in zero_shapes]
        out_arrs = sharded(*concat_in, *concat_zeros)
        jax.block_until_ready(out_arrs)
        return [
            {name: np.asarray(out_arrs[i]).reshape(n_cores, *out_avals[i].shape)[c]
             for i, name in enumerate(out_names)}
            for c in range(n_cores)
        ]

    return run


def _prep_inputs(features, captions, emb_table, W_ih, W_hh, b_ih, b_hh, fc_W, fc_b):
    features = np.asarray(features, dtype=np.float32)
    captions = np.asarray(captions)
    emb_table = np.asarray(emb_table, dtype=np.float32)
    W_ih = np.asarray(W_ih, dtype=np.float32)
    W_hh = np.asarray(W_hh, dtype=np.float32)
    b = (np.asarray(b_ih, dtype=np.float32) + np.asarray(b_hh, dtype=np.float32))
    fc_W = np.asarray(fc_W, dtype=np.float32)
    fc_b = np.asarray(fc_b, dtype=np.float32)

    embedded = emb_table[captions.astype(np.int64)]          # [B, T, EMB]
    lstm_in = np.concatenate([features, embedded], axis=-1)  # [B, T, DIN]

    w_ih_T = np.ascontiguousarray(W_ih.T.astype(ml_dtypes.bfloat16))
    w_hh_T = np.ascontiguousarray(W_hh.T.astype(ml_dtypes.bfloat16))
    bias_rep = np.ascontiguousarray(np.broadcast_to(b, (P, G)))

    in_maps = []
    for c in range(NCORES):
        xc = lstm_in[c * BL:(c + 1) * BL]                    # [BL, T, DIN]
        x_T = np.ascontiguousarray(
            xc.transpose(2, 1, 0).reshape(DIN, RL).astype(ml_dtypes.bfloat16))
        fc_wT = np.ascontiguousarray(
            fc_W[c * VL:(c + 1) * VL].T.astype(ml_dtypes.bfloat16))
        fcb_rep = np.ascontiguousarray(
            np.broadcast_to(fc_b[c * VL:(c + 1) * VL], (P, VL)))
        in_maps.append({
            "x_T": x_T, "w_ih_T": w_ih_T, "w_hh_T": w_hh_T, "bias_rep": bias_rep,
            "fc_wT": fc_wT, "fc_b_rep": fcb_rep,
        })
    return in_maps


def _row_perm():
    # device row r' -> (batch b_global, t); build gather index: perm[b*T+t] = r'
    perm = np.empty(B * T, dtype=np.int64)
    GT0 = [0, 4, 8]
    GSZ = [512, 512, 256]
    for rh in range(2):
        base_rh = rh * (RA // 2)
        for tg in range(3):
            ntg = 4 if tg < 2 else 2
            gbase = base_rh + sum(GSZ[:tg])
            for ap in range(4):
                for trem in range(ntg):
                    t = GT0[tg] + trem
                    for b in range(BL):
                        bg = (rh * 4 + ap) * BL + b
                        perm[bg * T + t] = gbase + ap * ntg * BL + trem * BL + b
    return perm


_PERM = _row_perm()


def _unshard(results):
    out = np.empty((B, T, VOCAB), dtype=np.float32)
    for c in range(NCORES):
        lg = results[c]["logits"][_PERM]                     # [B*T, VL]
        out[:, :, c * VL:(c + 1) * VL] = lg.reshape(B, T, VL)
    return out


def kernel(features, captions, emb_table, W_ih, W_hh, b_ih, b_hh, fc_W, fc_b):
    if "nc" not in _CACHE:
        _CACHE["nc"] = _build_nc()
    if "run" not in _CACHE:
        _CACHE["run"] = _build_sharded(_CACHE["nc"])
    in_maps = _prep_inputs(features, captions, emb_table, W_ih, W_hh, b_ih, b_hh,
                           fc_W, fc_b)
    results = _CACHE["run"](in_maps)
    return _unshard(results)


def kernel_traced(features, captions, emb_table, W_ih, W_hh, b_ih, b_hh, fc_W, fc_b):
    """Same computation via run_bass_kernel_spmd(trace=True); returns
    (output, BassKernelResults) so the caller can read exec_time_ns."""
    from concourse.bass_utils import run_bass_kernel_spmd
    if "nc" not in _CACHE:
        _CACHE["nc"] = _build_nc()
    in_maps = _prep_inputs(features, captions, emb_table, W_ih, W_hh, b_ih, b_hh,
                           fc_W, fc_b)
    res = run_bass_kernel_spmd(_CACHE["nc"], in_maps, list(range(NCORES)), trace=True)
    return _unshard(res.results), res


# revision 21
# speedup vs baseline: 1.0313x; 1.0313x over previous
"""Trainium2 Bass kernel for nn_Decoder (embed -> LSTM -> vocab projection).

v2 layout (8 NeuronCores, single SPMD NEFF):
  - Host: embedding gather + concat -> lstm_in; pre-transpose weights.
  - gx GEMM in rows-layout: gx[320,4096] = x @ W_ih^T + b (x stationary,
    W_ih^T moving, bias added on PSUM eviction).
  - LSTM recurrence, data-parallel over batch (32/core): gates[32,4096]
    computed with h_T as the stationary operand (64 N=512 matmuls/step) and
    the gx contribution folded in via an identity-matmul partition-select.
    ScalarE applies sigmoid/tanh straight from PSUM. h is re-transposed
    each step on the PE (h_T feeds the next step + the FC lhsT).
  - Per-timestep AllGather of h_T (runs on TOPSP, overlapped with compute).
  - FC vocab-sharded: logits[2560,3750] = hs @ fc_W_shard^T + fc_b, rows
    processed in 2 halves to fit SBUF; 1280 N<=512 fp32r matmuls.
  - Host: undo row permutation, concat vocab shards.

All matmuls are float32r (tf32-class, ~1.5e-4): raw fp32 bits are DMA'd
directly into float32r tensors (no on-device rounding pass needed).
"""
import ml_dtypes
import numpy as np
import jax
from jax.sharding import Mesh, PartitionSpec
from jax.experimental.shard_map import shard_map

import concourse.bass as bass
import concourse.mybir as mybir
import concourse.tile as tile
from concourse import bacc
from concourse.bass2jax import _bass_exec_p, install_neuronx_cc_hook, partition_id_tensor
from concourse.masks import make_identity

P = 128
NCORES = 8
B, T, FEAT, EMB, HID, VOCAB = 256, 10, 512, 512, 1024, 30000
DIN = FEAT + EMB          # 1024
G = 4 * HID               # 4096
BL = B // NCORES          # 32 batches per core
RL = BL * T               # 320 rows per core (t-major: r = t*BL + b)
RA = RL * NCORES          # 2560 rows total
VL = VOCAB // NCORES      # 3750 vocab per core
KT = DIN // P             # 8 contraction tiles
NCH = G // 512            # 8 gate column chunks
F32 = mybir.dt.float32
F32R = mybir.dt.float32r
BF16 = mybir.dt.bfloat16
Act = mybir.ActivationFunctionType

_CACHE = {}


def _build_nc():
    nc = bacc.Bacc("TRN2", target_bir_lowering=False, debug=False, num_devices=NCORES)
    x_T = nc.dram_tensor("x_T", [DIN, RL], BF16, kind="ExternalInput").ap()
    w_ih_T = nc.dram_tensor("w_ih_T", [DIN, G], BF16, kind="ExternalInput").ap()
    w_hh_T = nc.dram_tensor("w_hh_T", [HID, G], BF16, kind="ExternalInput").ap()
    bias_rep = nc.dram_tensor("bias_rep", [P, G], F32, kind="ExternalInput").ap()
    fc_wT = nc.dram_tensor("fc_wT", [HID, VL], BF16, kind="ExternalInput").ap()
    fc_b_rep = nc.dram_tensor("fc_b_rep", [P, VL], F32, kind="ExternalInput").ap()
    logits = nc.dram_tensor("logits", [RA, VL], F32, kind="ExternalOutput").ap()

    MT_X = [(0, 128), (128, 128), (256, 64)]  # (row0, rows) m-tiles of 320
    WINS = [(0, 1024), (1024, 1024), (2048, 1024), (3072, 678)]
    NORD = [0, 2, 4, 6, 1, 3, 5, 7]  # gate chunks: h-half0 first, then half1

    with tile.TileContext(nc) as tc:
        with tc.tile_pool(name="dram", bufs=1, space="DRAM") as dram_pool:
            hs_dram = dram_pool.tile([T, HID, BL], BF16)
            ag_outs = [dram_pool.tile([NCORES, HID, BL], BF16,
                                      addr_space="Shared", name=f"ag_{t}")
                       for t in range(T)]
            gx_dram = dram_pool.tile([3, P, G], BF16)

            with tc.tile_pool(name="persist", bufs=1) as persist, \
                 tc.tile_pool(name="phD", bufs=1) as phD, \
                 tc.tile_pool(name="hsT_pool", bufs=2) as hsT_pool, \
                 tc.tile_pool(name="fcw_pool", bufs=2) as fcw_pool, \
                 tc.tile_pool(name="fc_out", bufs=2) as fc_out:
                ident_f = persist.tile([P, P], F32)
                make_identity(nc, ident_f[:])
                ident_b = persist.tile([P, P], BF16)
                nc.vector.tensor_copy(ident_b[:], ident_f[:])
                gx_t0 = persist.tile([32, G], BF16)
                fcb_sb = phD.tile([P, VL], F32)
                GSZ = [512, 512, 256]   # rows per t-group (t0-3, t4-7, t8-9)
                GT0 = [0, 4, 8]
                hsT_tiles = {}
                for rh in range(2):
                    for g in range(3):
                        hsT_tiles[rh, g] = hsT_pool.tile(
                            [P, KT, GSZ[g]], BF16,
                            name=f"hsT_{rh}_{g}", tag=f"hsT_{g}")

                GB = [0, 512, 1024]

                def fc_block(rh, v0, vn, fw, g, ml, psum_pool, tagsfx=""):
                    row0 = rh * (RA // 2) + GB[g] + ml * P
                    hsT_sb = hsT_tiles[rh, g]
                    for n0 in range(0, vn, 512):
                        nsz = min(512, vn - n0)
                        ps = psum_pool.tile(
                            [P, 512], F32,
                            name=f"fps{tagsfx}_{rh}_{v0}_{g}_{ml}_{n0}",
                            tag=f"fps{tagsfx}", bufs=2 if tagsfx else None)
                        for k in range(KT):
                            nc.tensor.matmul(
                                ps[:, 0:nsz],
                                hsT_sb[:, k, ml * P:(ml + 1) * P],
                                fw[:, k, n0:n0 + nsz],
                                start=(k == 0), stop=(k == KT - 1))
                        ot = fc_out.tile(
                            [P, 512], F32,
                            name=f"fo_{rh}_{v0}_{g}_{ml}_{n0}", tag="fo")
                        nc.vector.tensor_add(
                            ot[:, 0:nsz], ps[:, 0:nsz],
                            fcb_sb[:, v0 + n0:v0 + n0 + nsz])
                        nc.sync.dma_start(
                            logits[row0:row0 + P, v0 + n0:v0 + n0 + nsz],
                            ot[:, 0:nsz])

                # ---- Phase A: gx = x @ W_ih^T + b   (rows x gates, bf16) ----
                with tc.tile_pool(name="phA", bufs=1) as phA, \
                     tc.tile_pool(name="wih_pool", bufs=3) as wih_pool, \
                     tc.tile_pool(name="gx_stage", bufs=4) as gx_stage, \
                     tc.tile_pool(name="gx_psum", bufs=4, space="PSUM") as gx_psum:
                    x_sb = phA.tile([P, KT, RL], BF16)
                    for k in range(KT):
                        nc.scalar.dma_start(
                            x_sb[:, k, :], x_T[k * P:(k + 1) * P, :])
                    bias_sb = phA.tile([P, G], F32)
                    nc.scalar.dma_start(bias_sb[:], bias_rep)
                    for n in range(NCH):
                        wt = wih_pool.tile([P, KT, 512], BF16,
                                           name=f"wih_{n}", tag="wih")
                        for kk in range(0, KT, 2):
                            nc.sync.dma_start(
                                wt[:, kk:kk + 2, :],
                                w_ih_T[:, n * 512:(n + 1) * 512].rearrange(
                                    "(k p) v -> p k v", p=P)[:, kk:kk + 2, :])
                        for mi, (r0, rn) in enumerate(MT_X):
                            ps = gx_psum.tile([P, 512], F32,
                                              name=f"gxps_{n}_{mi}", tag="gxps")
                            for k in range(KT):
                                nc.tensor.matmul(
                                    ps[0:rn, :], x_sb[:, k, r0:r0 + rn],
                                    wt[:, k, :],
                                    start=(k == 0), stop=(k == KT - 1))
                            gt = gx_stage.tile([P, 512], BF16,
                                               name=f"gxs_{n}_{mi}", tag="gxs")
                            nc.vector.tensor_add(
                                gt[0:rn, :],
                                ps[0:rn, :], bias_sb[0:rn, n * 512:(n + 1) * 512])
                            if mi == 0:
                                # keep t=0's rows on-chip (skips DRAM roundtrip)
                                nc.vector.tensor_copy(
                                    gx_t0[:, n * 512:(n + 1) * 512], gt[0:32, :])
                            nc.scalar.dma_start(
                                gx_dram[mi, 0:rn, n * 512:(n + 1) * 512], gt[0:rn, :])

                nc.scalar.dma_start(fcb_sb[:], fc_b_rep)

                # ---- Phase B: LSTM recurrence (bf16) ----
                with tc.tile_pool(name="phB", bufs=1) as phB, \
                     tc.tile_pool(name="whh_pool", bufs=1) as whh_pool, \
                     tc.tile_pool(name="gxb_pool", bufs=2) as gxb_pool, \
                     tc.tile_pool(name="gch_psum", bufs=4, space="PSUM") as gch_psum, \
                     tc.tile_pool(name="tp_psum", bufs=2, space="PSUM") as tp_psum, \
                     tc.tile_pool(name="step_pool", bufs=1) as step_pool:
                    whh_sb = whh_pool.tile([P, KT, G], BF16)
                    for n in range(NCH):
                        for kk in range(0, KT, 4):
                            nc.gpsimd.dma_start(
                                whh_sb[:, kk:kk + 4, n * 512:(n + 1) * 512],
                                w_hh_T[:, n * 512:(n + 1) * 512].rearrange(
                                    "(k p) v -> p k v", p=P)[:, kk:kk + 4, :])
                    h_Tb = phB.tile([P, KT, BL], BF16)
                    c_sb = phB.tile([32, HID], F32)
                    fw0 = fcw_pool.tile([P, KT, 1024], BF16, name="fcw_w0", tag="fcw")
                    for kk in range(0, KT, 4):
                        nc.sync.dma_start(
                            fw0[:, kk:kk + 4, :],
                            fc_wT[:, 0:1024].rearrange(
                                "(k p) v -> p k v", p=P)[:, kk:kk + 4, :])

                    for t in range(T):
                        mt, j = t // 4, t % 4
                        if t > 0:
                            gxt = gxb_pool.tile([32, G], BF16,
                                                name=f"gxt_{t}", tag="gxt")
                            nc.scalar.dma_start(
                                gxt[:], gx_dram[mt, 32 * j:32 * j + 32, :])
                        gates4 = step_pool.tile([32, 4, HID], F32,
                                                name=f"gates_{t}", tag="gates")
                        tmp = step_pool.tile([32, HID], F32, name=f"tmp_{t}", tag="tmp")
                        th = step_pool.tile([32, HID], F32, name=f"th_{t}", tag="tmp")
                        h_sb = step_pool.tile([32, HID], BF16, name=f"h_{t}", tag="h")

                        def gate_chunk(n):
                            g4, half = n // 2, n % 2
                            dst = gates4[:, g4, half * 512:(half + 1) * 512]
                            if t == 0:
                                nc.scalar.activation(
                                    dst, gx_t0[:, n * 512:(n + 1) * 512],
                                    Act.Tanh if g4 == 2 else Act.Sigmoid)
                                return
                            ps = gch_psum.tile([32, 512], F32,
                                               name=f"gps_{t}_{n}", tag="gps")
                            nc.tensor.matmul(
                                ps[:], ident_b[0:32, 0:32],
                                gxt[:, n * 512:(n + 1) * 512],
                                start=True, stop=False)
                            for k in range(KT):
                                nc.tensor.matmul(
                                    ps[:], h_Tb[:, k, :],
                                    whh_sb[:, k, n * 512:(n + 1) * 512],
                                    start=False, stop=(k == KT - 1))
                            nc.scalar.activation(
                                dst, ps[:], Act.Tanh if g4 == 2 else Act.Sigmoid)

                        def half_elemwise(half):
                            sl = slice(half * 512, (half + 1) * 512)
                            nc.vector.tensor_mul(tmp[:, sl], gates4[:, 0, sl],
                                                 gates4[:, 2, sl])
                            if t == 0:
                                nc.vector.tensor_copy(c_sb[:, sl], tmp[:, sl])
                            else:
                                nc.vector.tensor_mul(c_sb[:, sl], gates4[:, 1, sl],
                                                     c_sb[:, sl])
                                nc.vector.tensor_add(c_sb[:, sl], c_sb[:, sl],
                                                     tmp[:, sl])
                            nc.scalar.activation(th[:, sl], c_sb[:, sl], Act.Tanh)
                            nc.vector.tensor_mul(h_sb[:, sl], gates4[:, 3, sl],
                                                 th[:, sl])

                        for n in NORD[:4]:
                            gate_chunk(n)
                        half_elemwise(0)
                        for n in NORD[4:]:
                            gate_chunk(n)
                        half_elemwise(1)
                        for k in range(KT):
                            tp = tp_psum.tile([P, 32], BF16,
                                              name=f"tp_{t}_{k}", tag="tp")
                            nc.tensor.transpose(
                                tp[:], h_sb[:, k * P:(k + 1) * P], ident_b[0:32, 0:32])
                            nc.vector.tensor_copy(h_Tb[:, k, :], tp[:])
                        nc.scalar.dma_start(
                            hs_dram[t].rearrange("(k p) b -> p k b", p=P), h_Tb[:])
                        nc.gpsimd.collective_compute(
                            "AllGather", mybir.AluOpType.bypass,
                            replica_groups=[list(range(NCORES))],
                            ins=[hs_dram[t].opt()], outs=[ag_outs[t].opt()])
                        tg = 0 if t < 4 else (1 if t < 8 else 2)
                        ntg = 4 if tg < 2 else 2
                        for a in range(NCORES):
                            rh = a // 4
                            r0 = (a % 4) * ntg * BL + (t - GT0[tg]) * BL
                            nc.gpsimd.dma_start(
                                hsT_tiles[rh, tg][:, :, r0:r0 + BL],
                                ag_outs[t][a].rearrange("(k p) b -> p k b", p=P))
                        # backfill PE stalls with early FC work (fw0 resident)
                        ILV = {6: [(0, 0)], 7: [(0, 1)],
                               8: [(0, 2), (0, 3)], 9: [(1, 0), (1, 1)]}
                        for (g, ml) in ILV.get(t, []):
                            fc_block(0, 0, 1024, fw0, g, ml, gch_psum, tagsfx="i")

                # ---- Phase D: FC; t-groups 0/1 first, g2 (needs AG_9) last ----
                with tc.tile_pool(name="fc_psum", bufs=6, space="PSUM") as fc_psum:
                    def load_fw(tag_name, v0, vn):
                        fw = fcw_pool.tile([P, KT, 1024], BF16,
                                           name=tag_name, tag="fcw")
                        for kk in range(0, KT, 4):
                            nc.sync.dma_start(
                                fw[:, kk:kk + 4, 0:vn],
                                fc_wT[:, v0:v0 + vn].rearrange(
                                    "(k p) v -> p k v", p=P)[:, kk:kk + 4, :])
                        return fw

                    for rh in range(2):
                        for wi, (v0, vn) in enumerate(WINS):
                            fw = fw0 if (rh == 0 and wi == 0) \
                                else load_fw(f"fcw_{rh}_{v0}", v0, vn)
                            done = {(0, 0), (0, 1), (0, 2), (0, 3),
                                    (1, 0), (1, 1)} if (rh == 0 and wi == 0) else set()
                            for g in (0, 1):
                                for ml in range(4):
                                    if (g, ml) in done or (g == 1 and ml >= 4):
                                        continue
                                    fc_block(rh, v0, vn, fw, g, ml, fc_psum)
                    for rh in range(2):
                        for wi, (v0, vn) in enumerate(WINS):
                            fw = load_fw(f"fcwg2_{rh}_{v0}", v0, vn)
                            for ml in range(2):
                                fc_block(rh, v0, vn, fw, 2, ml, fc_psum)
    nc.compile()
    return nc


def _build_sharded(nc, n_cores=NCORES):
    install_neuronx_cc_hook()
    partition_name = nc.partition_id_tensor.name if nc.partition_id_tensor else None
    in_names, out_names, out_avals, zero_shapes = [], [], [], []
    for alloc in nc.m.functions[0].allocations:
        if not isinstance(alloc, mybir.MemoryLocationSet):
            continue
        name = alloc.memorylocations[0].name
        if alloc.kind == "ExternalInput":
            if name != partition_name:
                in_names.append(name)
        elif alloc.kind == "ExternalOutput":
            out_names.append(name)
            shape = tuple(alloc.tensor_shape)
            dtype = mybir.dt.np(alloc.dtype)
            out_avals.append(jax.core.ShapedArray(shape, dtype))
            zero_shapes.append((shape, dtype))
    n_params = len(in_names)
    n_outs = len(out_avals)
    all_in_names = list(in_names) + list(out_names)
    if partition_name is not None:
        all_in_names.append(partition_name)
    donate = tuple(range(n_params, n_params + n_outs))

    def _body(*args):
        operands = list(args)
        if partition_name is not None:
            operands.append(partition_id_tensor())
        outs = _bass_exec_p.bind(
            *operands,
            out_avals=tuple(out_avals),
            in_names=tuple(all_in_names),
            out_names=tuple(out_names),
            lowering_input_output_aliases=(),
            sim_require_finite=True,
            sim_require_nnan=True,
            nc=nc,
        )
        return tuple(outs)

    devices = jax.devices("axon")[:n_cores]
    mesh = Mesh(np.asarray(devices), ("core",))
    in_specs = (PartitionSpec("core"),) * (n_params + n_outs)
    out_specs = (PartitionSpec("core"),) * len(out_names)
    sharded = jax.jit(
        shard_map(_body, mesh=mesh, in_specs=in_specs, out_specs=out_specs,
                  check_rep=False),
        donate_argnums=donate, keep_unused=True)

    def run(in_maps):
        concat_in = [
            np.concatenate([np.asarray(m[name]) for m in in_maps], axis=0)
            for name in in_names
        ]
        concat_zeros = [np.zeros((n_cores * s[0], *s[1:]), d) for s, d in zero_shapes]
        out_arrs = sharded(*concat_in, *concat_zeros)
        jax.block_until_ready(out_arrs)
        return [
            {name: np.asarray(out_arrs[i]).reshape(n_cores, *out_avals[i].shape)[c]
             for i, name in enumerate(out_names)}
            for c in range(n_cores)
        ]

    return run


def _prep_inputs(features, captions, emb_table, W_ih, W_hh, b_ih, b_hh, fc_W, fc_b):
    features = np.asarray(features, dtype=np.float32)
    captions = np.asarray(captions)
    emb_table = np.asarray(emb_table, dtype=np.float32)
    W_ih = np.asarray(W_ih, dtype=np.float32)
    W_hh = np.asarray(W_hh, dtype=np.float32)
    b = (np.asarray(b_ih, dtype=np.float32) + np.asarray(b_hh, dtype=np.float32))
    fc_W = np.asarray(fc_W, dtype=np.float32)
    fc_b = np.asarray(fc_b, dtype=np.float32)

    embedded = emb_table[captions.astype(np.int64)]          # [B, T, EMB]
    lstm_in = np.concatenate([features, embedded], axis=-1)  # [B, T, DIN]

    w_ih_T = np.ascontiguousarray(W_ih.T.astype(ml_dtypes.bfloat16))
    w_hh_T = np.ascontiguousarray(W_hh.T.astype(ml_dtypes.bfloat16))
    bias_rep = np.ascontiguousarray(np.broadcast_to(b, (P, G)))

    in_maps = []
    for c in range(NCORES):
        xc = lstm_in[c * BL:(c + 1) * BL]                    # [BL, T, DIN]
        x_T = np.ascontiguousarray(
            xc.transpose(2, 1, 0).reshape(DIN, RL).astype(ml_dtypes.bfloat16))
        fc_wT = np.ascontiguousarray(
            fc_W[c * VL:(c + 1) * VL].T.astype(ml_dtypes.bfloat16))
        fcb_rep = np.ascontiguousarray(
            np.broadcast_to(fc_b[c * VL:(c + 1) * VL], (P, VL)))
        in_maps.append({
            "x_T": x_T, "w_ih_T": w_ih_T, "w_hh_T": w_hh_T, "bias_rep": bias_rep,
            "fc_wT": fc_wT, "fc_b_rep": fcb_rep,
        })
    return in_maps


def _row_perm():
    # device row r' -> (batch b_global, t); build gather index: perm[b*T+t] = r'
    perm = np.empty(B * T, dtype=np.int64)
    GT0 = [0, 4, 8]
    GSZ = [512, 512, 256]
    for rh in range(2):
        base_rh = rh * (RA // 2)
        for tg in range(3):
            ntg = 4 if tg < 2 else 2
            gbase = base_rh + sum(GSZ[:tg])
            for ap in range(4):
                for trem in range(ntg):
                    t = GT0[tg] + trem
                    for b in range(BL):
                        bg = (rh * 4 + ap) * BL + b
                        perm[bg * T + t] = gbase + ap * ntg * BL + trem * BL + b
    return perm


_PERM = _row_perm()


def _unshard(results):
    out = np.empty((B, T, VOCAB), dtype=np.float32)
    for c in range(NCORES):
        lg = results[c]["logits"][_PERM]                     # [B*T, VL]
        out[:, :, c * VL:(c + 1) * VL] = lg.reshape(B, T, VL)
    return out


def kernel(features, captions, emb_table, W_ih, W_hh, b_ih, b_hh, fc_W, fc_b):
    if "nc" not in _CACHE:
        _CACHE["nc"] = _build_nc()
    if "run" not in _CACHE:
        _CACHE["run"] = _build_sharded(_CACHE["nc"])
    in_maps = _prep_inputs(features, captions, emb_table, W_ih, W_hh, b_ih, b_hh,
                           fc_W, fc_b)
    results = _CACHE["run"](in_maps)
    return _unshard(results)


def kernel_traced(features, captions, emb_table, W_ih, W_hh, b_ih, b_hh, fc_W, fc_b):
    """Same computation via run_bass_kernel_spmd(trace=True); returns
    (output, BassKernelResults) so the caller can read exec_time_ns."""
    from concourse.bass_utils import run_bass_kernel_spmd
    if "nc" not in _CACHE:
        _CACHE["nc"] = _build_nc()
    in_maps = _prep_inputs(features, captions, emb_table, W_ih, W_hh, b_ih, b_hh,
                           fc_W, fc_b)
    res = run_bass_kernel_spmd(_CACHE["nc"], in_maps, list(range(NCORES)), trace=True)
    return _unshard(res.results), res
